# revision 6
# baseline (speedup 1.0000x reference)
"""DynamicLinear (MoE routing) Trainium2 Bass kernel.

Math (per sample b):
    out[b] = sum_k attn[b,k] * (x[b] @ W[k].T + bias[k])
           = sum_k attn[b,k] * (x[b] @ W[k].T) + attn[b] @ bias

Sharding: 8 cores in a 2x4 grid over (batch, out_features).
Each core computes out[b_half, o_quarter] from x[b_half] (16 MiB),
W[:, o_quarter, :] (16 MiB) -- no cross-core communication.

Per-core pipeline:
  1. gpsimd casting DMAs: x, W fp32 DRAM -> bf16 DRAM staging.
  2. xbar DMA transposes (bf16): W[k] [512,2048] -> wT[k] [128,16,512]
     (i on partitions); x group [512,2048] -> xT [128,16,512].
  3. TensorE: for each b_tile (16) x expert (4): accumulate 16 matmul
     passes (K=128 contraction, N=512 moving) into one PSUM bank.
  4. Combine on ACT+DVE: acc = sum_k attn[:,k]*(psum_k + bias[k]) with
     per-partition scalar multiply (attn lives on the b partition dim).
  5. DMA acc -> out.
"""

import numpy as np

_B, _K, _IN, _OUT = 4096, 4, 2048, 2048
_GRID_B, _GRID_O = 2, 4
_BL = _B // _GRID_B      # 2048 batch rows per core
_OL = _OUT // _GRID_O    # 512 out cols per core
_NBT = _BL // 128        # 16 b tiles
_NIT = _IN // 128        # 16 contraction tiles
_XG = 512                # batch rows per x-transpose group
_NG = _BL // _XG         # 4 groups

_CACHE = {}
LAST_RESULTS = None


def _build_program():
    import concourse.bass as bass
    import concourse.tile as tile
    from concourse import bacc, mybir

    f32 = mybir.dt.float32
    bf16 = mybir.dt.bfloat16
    MULT = mybir.AluOpType.mult
    ADD = mybir.AluOpType.add
    COPY = mybir.ActivationFunctionType.Copy

    nc = bacc.Bacc("TRN2", target_bir_lowering=False, debug=False)
    x = nc.dram_tensor("x", [_BL, _IN], f32, kind="ExternalInput").ap()
    attn = nc.dram_tensor("attn", [_BL, _K], f32, kind="ExternalInput").ap()
    w = nc.dram_tensor("w", [_K, _OL, _IN], f32, kind="ExternalInput").ap()
    bias = nc.dram_tensor("bias", [_K, _OL], f32, kind="ExternalInput").ap()
    out = nc.dram_tensor("out", [_BL, _OL], f32, kind="ExternalOutput").ap()

    with tile.TileContext(nc) as tc:
        with (
            tc.tile_pool(name="dram", bufs=1, space="DRAM") as dram,
            tc.tile_pool(name="wT", bufs=1) as wTp,
            tc.tile_pool(name="xT", bufs=2) as xTp,
            tc.tile_pool(name="singles", bufs=1) as singles,
            tc.tile_pool(name="attn", bufs=_NBT) as attnp,
            tc.tile_pool(name="acc", bufs=6) as accp,
            tc.tile_pool(name="psum", bufs=8, space="PSUM") as psump,
        ):
            # bias replicated across all 128 partitions: [128, K, OL]
            bias_rep = singles.tile([128, _K, _OL], f32)
            bias_bcast = bass.AP(
                tensor=bias.tensor,
                offset=bias.offset,
                ap=[[0, 128], bias.ap[0], bias.ap[1]],
            )
            nc.gpsimd.dma_start(out=bias_rep, in_=bias_bcast)

            # attn tiles, b on partitions: [128, K] per b_tile
            attn_sb = []
            for t in range(_NBT):
                a = attnp.tile([128, _K], f32, tag="attn")
                nc.sync.dma_start(out=a, in_=attn[t * 128:(t + 1) * 128, :])
                attn_sb.append(a)

            # Phase 0: fp32 -> bf16 casting DMAs into DRAM staging
            wbf = dram.tile([_K, _OL, _IN], bf16)
            for k in range(_K):
                nc.gpsimd.dma_start(out=wbf[k], in_=w[k])
            xbf = dram.tile([_BL, _IN], bf16)
            for g in range(_NG):
                nc.gpsimd.dma_start(
                    out=xbf[g * _XG:(g + 1) * _XG, :],
                    in_=x[g * _XG:(g + 1) * _XG, :],
                )

            # Phase 1: xbar transposes DRAM -> SBUF
            # wT[k][i_in, ii, o] = W[k][o, ii*128 + i_in]
            wT = []
            for k in range(_K):
                wt = wTp.tile([128, _NIT, _OL], bf16, tag=f"wT{k}")
                nc.sync.dma_start_transpose(wt, wbf[k])
                wT.append(wt)

            # Phase 2: matmuls + combine, one 128-row b_tile at a time
            xT = {}
            for t in range(_NBT):
                g, bq = divmod(t, _XG // 128)
                if bq == 0:
                    # xT[g][i_in, ii, b] = x[g*XG + b, ii*128 + i_in]
                    xT[g] = xTp.tile([128, _NIT, _XG], bf16, tag="xT",
                                     name=f"xT{g}")
                    nc.sync.dma_start_transpose(
                        xT[g], xbf[g * _XG:(g + 1) * _XG, :]
                    )

                ps = [psump.tile([128, _OL], mybir.dt.float32, tag="ps",
                                 name=f"ps_t{t}_k{k}")
                      for k in range(_K)]
                for ii in range(_NIT):
                    lhsT = xT[g][:, ii, bq * 128:(bq + 1) * 128]
                    for k in range(_K):
                        nc.tensor.matmul(
                            ps[k], lhsT=lhsT, rhs=wT[k][:, ii, :],
                            start=(ii == 0), stop=(ii == _NIT - 1),
                        )

                # combine: acc = sum_k attn_k * (bias_k + psum_k)
                a_t = attn_sb[t]
                acc0 = accp.tile([128, _OL], mybir.dt.float32, tag="acc")
                acc1 = accp.tile([128, _OL], mybir.dt.float32, tag="acc")
                nc.scalar.activation(
                    acc0, bias_rep[:, 0, :], COPY, scale=a_t[:, 0:1]
                )
                cur, nxt = acc0, acc1
                for k in range(1, _K):
                    nc.vector.scalar_tensor_tensor(
                        out=nxt, in0=bias_rep[:, k, :], scalar=a_t[:, k:k + 1],
                        in1=cur, op0=MULT, op1=ADD,
                    )
                    cur, nxt = nxt, cur
                for k in range(_K):
                    nc.vector.scalar_tensor_tensor(
                        out=nxt, in0=ps[k], scalar=a_t[:, k:k + 1],
                        in1=cur, op0=MULT, op1=ADD,
                    )
                    cur, nxt = nxt, cur
                nc.sync.dma_start(out=out[t * 128:(t + 1) * 128, :], in_=cur)

    nc.compile()
    return nc


def _get_program():
    if "nc" not in _CACHE:
        _CACHE["nc"] = _build_program()
    return _CACHE["nc"]


def _ensure_axon_hooks_importable():
    """bass_utils' trace branch imports antenv.axon_hooks, which the
    trimmed agent image may lack; stub it (hook=None) so a stray
    BASS_TRACE=1 degrades to an untraced run instead of crashing."""
    import sys
    import types

    try:
        import antenv.axon_hooks  # noqa: F401
        return
    except ImportError:
        pass
    mod = types.ModuleType("antenv.axon_hooks")
    mod._hook = None
    mod.get_axon_ntff_profile_hook = lambda: mod._hook

    def _set(h):
        mod._hook = h

    mod.set_axon_ntff_profile_hook = _set
    sys.modules["antenv.axon_hooks"] = mod
    try:
        import antenv
        antenv.axon_hooks = mod
    except ImportError:
        pass


def kernel(**inputs):
    global LAST_RESULTS
    from concourse.bass_utils import run_bass_kernel_spmd

    _ensure_axon_hooks_importable()

    x = np.ascontiguousarray(inputs["x"], dtype=np.float32)
    attn = np.ascontiguousarray(inputs["softmax_attention"], dtype=np.float32)
    w = np.ascontiguousarray(inputs["weight"], dtype=np.float32)
    b = np.ascontiguousarray(inputs["bias"], dtype=np.float32)

    nc = _get_program()
    in_maps = []
    for c in range(8):
        gb, go = divmod(c, _GRID_O)
        in_maps.append({
            "x": np.ascontiguousarray(x[gb * _BL:(gb + 1) * _BL]),
            "attn": np.ascontiguousarray(attn[gb * _BL:(gb + 1) * _BL]),
            "w": np.ascontiguousarray(w[:, go * _OL:(go + 1) * _OL, :]),
            "bias": np.ascontiguousarray(b[:, go * _OL:(go + 1) * _OL]),
        })

    res = run_bass_kernel_spmd(nc, in_maps, list(range(8)))
    LAST_RESULTS = res

    full = np.empty((_B, _OUT), dtype=np.float32)
    for c in range(8):
        gb, go = divmod(c, _GRID_O)
        full[gb * _BL:(gb + 1) * _BL, go * _OL:(go + 1) * _OL] = \
            res.results[c]["out"]
    return full
